# revision 59
# baseline (speedup 1.0000x reference)
"""Multi-head attention (B=4, S=2048, D=768, H=12) on 8 Trainium2 cores.

Sharding: core c handles batch b=c//2 and head-half hh=c%2 (6 of 12 heads).
Each core computes its 6 heads' contribution to out[b] = concat(O_h) @ Wo
as a partial product; the host sums the two half-head partials per batch.

Device-side layout is feature-major ("T") for q/k activations so that no
on-device transposes are needed:
  - qhT = (Wq.T @ q.T): matmul(lhsT=Wq tile, rhs=qT tile) -> [d_model, S]
    (all bf16: softmax output noise is PROPORTIONAL to score noise --
    fp8 scores measure ~5.5%% output error, over the 2e-2 gate -- so the
    whole q/k path stays bf16; the 1/sqrt(64) attention scale rides the
    exp() activation's free scale operand.)
  - S^T scores: matmul(lhsT=khT head tile, rhs=qhT head tile) -> [S_k, S_q]
    (two heads packed in the 128-row PE array: K=64 each, rows 0:64/64:128)
  - softmax: exp on ScalarE direct PSUM->SBUF (bf16); the k-sum (softmax
    denominator) comes free from a ones-column folded into the PV matmul
    stationary operand (M=65); no max-subtraction (logits are O(10) here,
    exp is safe in fp32 and the harness reference uses the same math).
  - PV: matmul(lhsT=[vh|1] tile, rhs=E^T tile) accumulated over S_k -> O^T
  - normalize: DVE copy drains the PV accumulator (frees the bank), a DMA
    shifts the denominator row to partition 0, DVE approx-reciprocal,
    GpSimd partition-broadcast, DVE multiply (+ DMA shift for the odd head).
  - out = (O^T).T @ Wo tiles -> seq-major [S, 768] partial, DMA'd out.

The ScalarE exp stream (192 x ~1.11us = 214us) is the pacing engine; the
schedule's job is to keep it gapless:
  - input DMA (~34us at the ~350GB/s per-core HBM limit) is issued in
    consumption order with host-side layouts that make every line >=4.6KB
    contiguous per partition; warm-up zero-matmuls cover the DMA waits so
    the HAM clock gate keeps the PE at 2.4GHz through the lead-in.
  - each chunk emits scores+exp first; PV lags one k-tile and the previous
    chunk's last PV + normalize ride a carry into the next chunk's slots,
    so stragglers never block the exp stream (the PE executes in order).
  - v-projections are split per head-pair (hp0's slice just-in-time in
    chunk0, hp1's in hp0's later chunks, hp2's in hp1's) and the first two
    chunks of hp0/hp1 are FUSED: qc1's first exps are borrowed into qc0's
    window and its PVs deferred to a compressed post-window, paying qc0's
    PE oversubscription out of qc1's exp-time surplus.
  - projections double-buffer one PSUM bank (ps_m x2) -- PSUM budget is
    scores 2x2 + PV 2 + proj 2 = 8 banks exactly -- which lets adjacent
    projection groups overlap their drain CASTs.
  - the final out-projection rotates over all free PSUM tags, alternates
    its drains between DVE and the (now idle) ScalarE, and the last
    chunk's normalize interleaves both halves' engine chains.
NOTE: the Tile scheduler is dependency+priority driven -- dependency-free
instructions (e.g. warm-up matmuls) get hoisted to kernel start no matter
where they are emitted; ordering tricks only work through data deps.
"""

import sys
import types

import numpy as np
import ml_dtypes

import concourse.bacc as bacc
import concourse.mybir as mybir
import concourse.tile as tile

BF16 = mybir.dt.bfloat16
FP32 = mybir.dt.float32
FP8 = mybir.dt.float8e4

B, S, D, H = 4, 2048, 768, 12
DH = 64          # head dim
HPC = 6          # heads per core
DPC = HPC * DH   # feature columns per core (384)
P = 128
KT = D // P      # 6 contraction tiles for projections
ST = S // P      # 16 seq tiles
NCORES = 8
EXP_SCALE = 1.0 / np.sqrt(DH)   # attention scale, applied for free by exp()


def _install_ntff_hook_shim():
    """The image's antenv lacks axon_hooks; provide it so trace=True works."""
    if "antenv.axon_hooks" in sys.modules:
        return
    mod = types.ModuleType("antenv.axon_hooks")
    _hook = [None]
    mod.set_axon_ntff_profile_hook = lambda h: _hook.__setitem__(0, h)
    mod.get_axon_ntff_profile_hook = lambda: _hook[0]
    sys.modules["antenv.axon_hooks"] = mod
    try:
        import antenv

        antenv.axon_hooks = mod
    except ImportError:
        pass
    try:
        from trn_agent_boot.trn_boot import _ntff_profile_via_ctypes

        mod.set_axon_ntff_profile_hook(
            _ntff_profile_via_ctypes("/opt/axon/libaxon_pjrt.so")
        )
    except Exception:
        pass


_install_ntff_hook_shim()


def build_kernel():
    nc = bacc.Bacc("TRN2", target_bir_lowering=False, debug=True)
    # activations staged host-side as [quarter, partition, ktile, col] and
    # weights as [partition, ktile, cols] so every DMA line is contiguous
    # per partition (6KB lines instead of 1KB gather lines).
    d_qT = nc.declare_dram_parameter("qT", [4, P, KT, S // 4], BF16, isOutput=False)
    d_kT = nc.declare_dram_parameter("kT", [4, P, KT, S // 4], BF16, isOutput=False)
    d_vT = nc.declare_dram_parameter("vT", [4, P, KT, S // 4], BF16, isOutput=False)
    d_wq = nc.declare_dram_parameter("wq", [P, KT, DPC], BF16, isOutput=False)
    d_wk = nc.declare_dram_parameter("wk", [P, KT, DPC], BF16, isOutput=False)
    d_wv = nc.declare_dram_parameter("wv", [P, KT, DPC], BF16, isOutput=False)
    d_wo = nc.declare_dram_parameter("wo", [P, HPC // 2, D], BF16, isOutput=False)
    d_out = nc.declare_dram_parameter("out", [S, D], BF16, isOutput=True)

    with tile.TileContext(nc) as tc:
        persist_cm = tc.tile_pool(name="persist", bufs=1)
        pp = persist_cm.__enter__()

        # --- persistent SBUF inputs ---
        sb_qT = pp.tile([P, KT, S], BF16, tag="sb_qT")
        sb_kT = pp.tile([P, KT, S], BF16, tag="sb_kT")
        sb_vT = pp.tile([P, KT, S], BF16, tag="sb_vT")
        sb_wq = pp.tile([P, KT, DPC], BF16, tag="sb_wq")
        sb_wk = pp.tile([P, KT, DPC], BF16, tag="sb_wk")
        sb_wv = pp.tile([P, KT, DPC], BF16, tag="sb_wv")
        sb_wo = pp.tile([P, HPC // 2, D], BF16, tag="sb_wo")
        warm_sb = pp.tile([P, 512], BF16, tag="warm_sb")
        ones_pb = pp.tile([P, DH], FP32, tag="ones_pb")
        nc.vector.memset(ones_pb, 1.0)

        # PE warm-up: the HAM clock gate needs ~3.4us of sustained matmul
        # activity to lift the PE from 1.2 to 2.4 GHz.  Burn it on zero
        # matmuls while the first input DMAs are still in flight.
        nc.gpsimd.memset(warm_sb, 0.0)
        psum_cm = tc.tile_pool(name="ps", bufs=1, space="PSUM")
        psm = psum_cm.__enter__()
        warm_ps = psm.tile([P, 2, 512], FP32, tag="ps_s", name="warm_ps", bufs=2)
        for i in range(16):
            nc.tensor.matmul(
                warm_ps[:, i % 2, :], warm_sb[:, 0:P], warm_sb, start=True, stop=True
            )

        # --- input DMA in consumption order ---
        # Weights first (single launches, >=512B lines), then the first
        # column chunk of kT/qT/vT so projections can start, then the rest
        # of kT/vT interleaved (chunk (hp0,qc0) consumes ALL of kh and vh),
        # qT trailing (chunk qc only needs qh columns qc*512+), wo last.
        def dma_w(sb, dr):
            nc.sync.dma_start(out=sb, in_=dr[:, :, :])

        def dma_cols(sb, dr, i):
            w = S // 4
            nc.sync.dma_start(
                out=sb[:, :, i * w : (i + 1) * w], in_=dr[i, :, :, :]
            )

        # critical path to the FIRST exp: wk+kT0 (k-proj sc0), wq+qT0
        # (q-proj sc0) -- everything else trails in consumption order.
        dma_w(sb_wk, d_wk)
        dma_cols(sb_kT, d_kT, 0)
        dma_w(sb_wq, d_wq)
        dma_cols(sb_qT, d_qT, 0)
        dma_w(sb_wv, d_wv)
        dma_cols(sb_vT, d_vT, 0)
        dma_cols(sb_kT, d_kT, 1)
        dma_cols(sb_vT, d_vT, 1)
        dma_cols(sb_qT, d_qT, 1)
        dma_cols(sb_kT, d_kT, 2)
        dma_cols(sb_vT, d_vT, 2)
        dma_cols(sb_kT, d_kT, 3)
        dma_cols(sb_vT, d_vT, 3)
        dma_cols(sb_qT, d_qT, 2)
        dma_cols(sb_qT, d_qT, 3)
        nc.sync.dma_start(out=sb_wo, in_=d_wo[:, :, :])

        QC = 512           # q positions per attention chunk
        NQ = S // QC       # 4 chunks
        NC2 = D // 2       # output projection n-halves (one PSUM bank each)
        NHP = HPC // 2     # 3 head pairs
        FAR = 10**9

        # --- persistent activations ---
        sb_qh = [pp.tile([P, S], BF16, tag=f"sb_qh{i}", name=f"sb_qh{i}") for i in range(NHP)]
        sb_kh = [pp.tile([P, S], BF16, tag=f"sb_kh{i}", name=f"sb_kh{i}") for i in range(NHP)]
        sb_vh = [
            pp.tile([P, HPC, DH + 1], BF16, tag=f"sb_vh{i}", name=f"sb_vh{i}")
            for i in range(ST)
        ]  # [v | 1] per seq tile
        sb_o = [
            [pp.tile([P, QC], BF16, tag=f"sb_o{i}_{j}", name=f"sb_o{i}_{j}") for j in range(NQ)]
            for i in range(NHP)
        ]
        for i in range(ST):
            nc.vector.memset(sb_vh[i][:, :, DH : DH + 1], 1.0)

        sb_cm = tc.tile_pool(name="work", bufs=1)
        wk = sb_cm.__enter__()

        def v_proj_closures(st, part):
            """part = head-pair: hp0's slice is needed in chunk0, hp1's
            weaves into hp0's chunks 1-3, hp2's into hp1's chunks 1-3.
            Splitting keeps each chunk's extra PE load bounded."""
            c0, c1 = 2 * DH * part, 2 * DH * (part + 1)
            nh = (c1 - c0) // DH
            stt = {}
            out = []
            for kt in range(KT):
                def mm(kt=kt):
                    if kt == 0:
                        stt["ps"] = psm.tile(
                            [P, c1 - c0], FP32, tag="ps_m", name="ps_v"
                        )
                    nc.tensor.matmul(
                        stt["ps"],
                        sb_vT[:, kt, st * P : (st + 1) * P],
                        sb_wv[:, kt, c0:c1],
                        start=(kt == 0),
                        stop=(kt == KT - 1),
                    )
                    if kt == KT - 1:
                        nc.vector.tensor_copy(
                            out=sb_vh[st][:, c0 // DH : c1 // DH, 0:DH],
                            in_=stt["ps"][:].rearrange("p (h d) -> p h d", h=nh),
                        )
                out.append(mm)
            return out

        def qk_proj_closures(hp, which, sc):
            """One closure per PE instruction so the group can be woven
            between attention k-tiles (PE executes its stream in order)."""
            sb_w, sb_x, dst = (
                (sb_wq, sb_qT, sb_qh[hp]) if which == "q" else (sb_wk, sb_kT, sb_kh[hp])
            )
            st = {}
            out = []
            for kt in range(KT):
                def mm(kt=kt):
                    if kt == 0:
                        st["ps"] = psm.tile([P, 512], FP32, tag="ps_m", name="ps_qk")
                    nc.tensor.matmul(
                        st["ps"],
                        sb_w[:, kt, hp * P : (hp + 1) * P],
                        sb_x[:, kt, sc * 512 : (sc + 1) * 512],
                        start=(kt == 0),
                        stop=(kt == KT - 1),
                    )
                    if kt == KT - 1:
                        nc.vector.tensor_copy(
                            out=dst[:, sc * 512 : (sc + 1) * 512], in_=st["ps"]
                        )
                out.append(mm)
            return out

        def out_proj_closures(qt, tags=("ps_m", "ps_m"), drains=("vector", "vector")):
            qc, qr = qt // (QC // P), qt % (QC // P)
            st = {}
            out = []
            for n2 in range(2):
                for hp in range(NHP):
                    def mm(n2=n2, hp=hp):
                        if hp == 0 and n2 == 0:
                            st["outt"] = wk.tile([P, D], BF16, tag="outt", bufs=4, name="outt")
                        if hp == 0:
                            bufs = {"ps_pv": 2, "ps_s": 2, "ps_m": 2}[tags[n2]]
                            st["ps"] = psm.tile(
                                [P, NC2], FP32, tag=tags[n2], name="ps_o", bufs=bufs
                            )
                        nc.tensor.matmul(
                            st["ps"],
                            sb_o[hp][qc][:, qr * P : (qr + 1) * P],
                            sb_wo[:, hp, n2 * NC2 : (n2 + 1) * NC2],
                            start=(hp == 0),
                            stop=(hp == NHP - 1),
                        )
                        if hp == NHP - 1:
                            if drains[n2] == "scalar":
                                # ScalarE is idle after the last exp; let it
                                # drain half the tail psum groups in parallel
                                nc.scalar.copy(
                                    out=st["outt"][:, n2 * NC2 : (n2 + 1) * NC2],
                                    in_=st["ps"],
                                )
                            else:
                                nc.vector.tensor_copy(
                                    out=st["outt"][:, n2 * NC2 : (n2 + 1) * NC2],
                                    in_=st["ps"],
                                )
                            # ship each half as soon as it is copied
                            nc.sync.dma_start(
                                out=d_out[
                                    qt * P : (qt + 1) * P,
                                    n2 * NC2 : (n2 + 1) * NC2,
                                ],
                                in_=st["outt"][:, n2 * NC2 : (n2 + 1) * NC2],
                            )
                    out.append(mm)
            return out

        def alloc_ps_pv():
            return [
                psm.tile([P, QC], FP32, tag="ps_pv", name="ps_pv_e", bufs=2),
                psm.tile([P, QC], FP32, tag="ps_pv", name="ps_pv_o", bufs=2),
            ]

        def scores_exp(hp, qc, kt):
            ps_s = psm.tile([P, 2, QC], FP32, tag="ps_s", name="ps_s", bufs=2)
            for h01 in range(2):
                hs = slice(DH * h01, DH * (h01 + 1))
                nc.tensor.matmul(
                    ps_s[:, h01, :],
                    sb_kh[hp][hs, kt * P : kt * P + P],
                    sb_qh[hp][hs, qc * QC : qc * QC + QC],
                    start=True,
                    stop=True,
                    tile_position=(DH * h01, 0),
                )
            e_t = wk.tile([P, 2, QC], BF16, tag="e_t", bufs=12, name="e_t")
            nc.scalar.activation(
                out=e_t,
                in_=ps_s,
                func=mybir.ActivationFunctionType.Exp,
                scale=float(EXP_SCALE),
            )
            return e_t

        def pv_pair(ps_pv, hp, kt, e_t):
            for h01 in range(2):
                h = hp * 2 + h01
                nc.tensor.matmul(
                    ps_pv[h01][0 : DH + 1, :],
                    sb_vh[kt][:, h, :],
                    e_t[:, h01, :],
                    start=(kt == 0),
                    stop=(kt == ST - 1),
                )

        def normalize_half(ps_pv, hp, qc, h01):
            # normalize: O^T[d, q] / denom[q]; denom sits at PSUM row DH.
            # One full copy off PSUM releases the accumulator slot early; the
            # rest of the chain works from SBUF off the critical path.
            o_un = wk.tile([DH + 1, QC], FP32, tag="o_un", bufs=4, name="o_un")
            nc.vector.tensor_copy(out=o_un, in_=ps_pv[h01][0 : DH + 1, :])
            rts = wk.tile([1, QC], FP32, tag="rts", bufs=2, name="rts")
            nc.sync.dma_start(out=rts, in_=o_un[DH : DH + 1, :])
            rt0 = wk.tile([1, QC], FP32, tag="rt0", bufs=2, name="rt0")
            nc.vector.reciprocal_approx_fast(out=rt0, in_=rts)
            bcr = wk.tile([DH, QC], FP32, tag="bcr", bufs=2, name="bcr")
            nc.gpsimd.partition_broadcast(bcr, rt0, channels=DH)
            if h01 == 0:
                nc.vector.tensor_mul(
                    out=sb_o[hp][qc][0:DH, :], in0=o_un[0:DH, :], in1=bcr
                )
            else:
                # odd head belongs at partitions 64:128 of the pair-packed
                # O^T; DVE can't cross lanes, so temp tile + DMA shift.
                o_tmp = wk.tile([DH, QC], BF16, tag="o_tmp", bufs=2, name="o_tmp")
                nc.vector.tensor_mul(out=o_tmp, in0=o_un[0:DH, :], in1=bcr)
                nc.sync.dma_start(out=sb_o[hp][qc][DH:P, :], in_=o_tmp)

        def normalize_both(ps_pv, hp, qc):
            """Interleaved both-halves normalize for the LAST chunk: the two
            chains' DVE/gpsimd/DMA stages alternate so neither engine queue
            serializes the other half behind it (~1.7us faster than two
            sequential normalize_half calls)."""
            o_un_e = wk.tile([DH + 1, QC], FP32, tag="o_un", bufs=4, name="o_un")
            nc.vector.tensor_copy(out=o_un_e, in_=ps_pv[0][0 : DH + 1, :])
            o_un_o = wk.tile([DH + 1, QC], FP32, tag="o_un", bufs=4, name="o_un")
            nc.vector.tensor_copy(out=o_un_o, in_=ps_pv[1][0 : DH + 1, :])
            rts_e = wk.tile([1, QC], FP32, tag="rts", bufs=2, name="rts")
            nc.sync.dma_start(out=rts_e, in_=o_un_e[DH : DH + 1, :])
            rts_o = wk.tile([1, QC], FP32, tag="rts", bufs=2, name="rts")
            nc.sync.dma_start(out=rts_o, in_=o_un_o[DH : DH + 1, :])
            rt0_e = wk.tile([1, QC], FP32, tag="rt0", bufs=2, name="rt0")
            nc.vector.reciprocal_approx_fast(out=rt0_e, in_=rts_e)
            rt0_o = wk.tile([1, QC], FP32, tag="rt0", bufs=2, name="rt0")
            nc.vector.reciprocal_approx_fast(out=rt0_o, in_=rts_o)
            bcr_e = wk.tile([DH, QC], FP32, tag="bcr", bufs=2, name="bcr")
            nc.gpsimd.partition_broadcast(bcr_e, rt0_e, channels=DH)
            bcr_o = wk.tile([DH, QC], FP32, tag="bcr", bufs=2, name="bcr")
            nc.gpsimd.partition_broadcast(bcr_o, rt0_o, channels=DH)
            nc.vector.tensor_mul(
                out=sb_o[hp][qc][0:DH, :], in0=o_un_e[0:DH, :], in1=bcr_e
            )
            o_tmp = wk.tile([DH, QC], BF16, tag="o_tmp", bufs=2, name="o_tmp")
            nc.vector.tensor_mul(out=o_tmp, in0=o_un_o[0:DH, :], in1=bcr_o)
            nc.sync.dma_start(out=sb_o[hp][qc][DH:P, :], in_=o_tmp)

        def attention_chunk(hp, qc, extras, carry, last=False):
            """Emits this chunk's scores/exp stream; the previous chunk's
            last PV + normalize halves arrive via `carry` and are emitted
            after this chunk's first exps so the exp stream never waits on
            them. Returns this chunk's own carry list."""
            ps_pv = alloc_ps_pv()
            pv_prev = [None]
            for kt in range(ST):
                gkt = qc * ST + kt
                e_t = scores_exp(hp, qc, kt)
                # previous chunk's straggler work (its last PV + normalize),
                # one piece per slot right after the exp is queued
                if carry:
                    carry.pop(0)()
                # deadline-due producers emit AFTER this slot's exp, so a
                # stalled producer (DMA-gated projection) never delays the
                # exp stream; their readers are >=1 slot downstream.
                while extras and extras[0][0] <= gkt:
                    extras.pop(0)[1]()
                # PV lags scores/exp by one k-tile: a late vh[kt] write
                # stalls only the (off-critical-path) accumulate, not exp.
                if pv_prev[0] is not None:
                    pv_prev[0]()
                pv_prev[0] = lambda kt=kt, e=e_t: pv_pair(ps_pv, hp, kt, e)
                # weave in foreign PE work (next projections / output proj)
                # where the PE has slack relative to the exp stream
                budget = 1
                while extras and budget > 0:
                    extras.pop(0)[1]()
                    budget -= 1
            if last:
                def last_pv_and_norm(f=pv_prev[0]):
                    f()
                    normalize_both(ps_pv, hp, qc)

                return [last_pv_and_norm]

            def last_pv_and_norm_e(f=pv_prev[0]):
                f()
                normalize_half(ps_pv, hp, qc, 0)

            return [
                last_pv_and_norm_e,
                lambda: normalize_half(ps_pv, hp, qc, 1),
            ]

        BOR = 7

        def fused_chunks01(hp, extras, carry):
            """qc0+qc1 fused: qc1's first BOR exps are emitted inside qc0's
            slots ST-BOR..ST-1 and its remaining exps + ALL its PVs in a
            compressed post window. qc0's PE oversubscription (v-proj +
            k-proj + its own attention exceed its exp time) is paid from
            qc1's exp-time surplus instead of stalling the exp stream."""
            ps_pv0 = alloc_ps_pv()
            e1 = []
            pv_prev = [None]
            for kt in range(ST):
                e_t0 = scores_exp(hp, 0, kt)
                if carry:
                    carry.pop(0)()
                while extras and extras[0][0] <= kt:
                    extras.pop(0)[1]()
                if pv_prev[0] is not None:
                    pv_prev[0]()
                pv_prev[0] = lambda kt=kt, e=e_t0: pv_pair(ps_pv0, hp, kt, e)
                if kt >= ST - BOR:
                    e1.append(scores_exp(hp, 1, kt - (ST - BOR)))
                budget = 1
                while extras and budget > 0:
                    extras.pop(0)[1]()
                    budget -= 1
            def last_pv0_and_norm_e(f=pv_prev[0]):
                f()
                normalize_half(ps_pv0, hp, 0, 0)

            carry0 = [
                last_pv0_and_norm_e,
                lambda: normalize_half(ps_pv0, hp, 0, 1),
            ]
            ps_pv1 = alloc_ps_pv()
            pend = 0
            for s in range(ST - BOR):
                kt = BOR + s
                e1.append(scores_exp(hp, 1, kt))
                if carry0:
                    carry0.pop(0)()
                gk = ST + 2 * s
                while extras and extras[0][0] <= gk:
                    extras.pop(0)[1]()
                for _ in range(2):
                    if pend < ST and pend <= kt and pend < len(e1):
                        pv_pair(ps_pv1, hp, pend, e1[pend])
                        pend += 1
            while pend < ST:
                pv_pair(ps_pv1, hp, pend, e1[pend])
                pend += 1
            while carry0:
                carry0.pop(0)()
            return [
                lambda: normalize_half(ps_pv1, hp, 1, 0),
                lambda: normalize_half(ps_pv1, hp, 1, 1),
            ]

        # --- schedule: front-load only what attention chunk (0,0) needs at
        # its very first k-tiles (early v-proj tiles, first k/q projection
        # chunks of hp0); everything else is woven into the attention chunks'
        # PE slack with deadlines that keep writers ahead of their readers.
        for cl in qk_proj_closures(0, "k", 0):
            cl()
        # more zero matmuls between k-proj (kT0-gated) and q-proj
        # (qT0-gated): they fill the DMA wait so the HAM clock gate never
        # sees a >3.4us PE idle and q-proj runs at 2.4GHz, not 1.2
        for i in range(8):
            nc.tensor.matmul(
                warm_ps[:, i % 2, :], warm_sb[:, 0:P], warm_sb, start=True, stop=True
            )
        for cl in qk_proj_closures(0, "q", 0):
            cl()
        # groups woven into each hp's chunks, each kept contiguous in list
        # order (they share one PSUM slot); deadlines are local k-tile slots
        hp_groups = [[] for _ in range(NHP)]
        for st in range(ST):
            # the vh[st] copy (last closure) must land by slot st+1 where
            # PV(st) reads it; hp0's 2-head slice only
            hp_groups[0].append((st - 4, v_proj_closures(st, 0)))
            # hp1's 2-head slice: first half rides hp0's chunks 1-3 slack,
            # second half lands just-in-time in hp1's own chunk0 (hp0 is
            # PE-oversubscribed; hp1 has slack). hp2's slice rides hp1's
            # chunks 1-3.
            if st < 8:
                hp_groups[0].append((16 + 4 * st, v_proj_closures(st, 1)))
            else:
                hp_groups[1].append((st - 4, v_proj_closures(st, 1)))
            hp_groups[1].append((18 + 2 * st, v_proj_closures(st, 2)))
        for sc in range(1, 4):
            hp_groups[0].append((4 * sc - 6, qk_proj_closures(0, "k", sc)))
            # q sc1 must finish by slot ST-BOR-1 where the fused chunk0
            # starts borrowing qc1's scores (qT1 has landed by ~slot 4)
            qdl = 4 if sc == 1 else sc * ST - 6
            hp_groups[0].append((qdl, qk_proj_closures(0, "q", sc)))
        for hp in range(1, NHP):
            # k sc1-3 are due just before this hp's chunk0 reads kt=4sc;
            # q sc1 early (fused chunk0 borrows qc1 scores from slot ST-BOR),
            # q sc2-3 before its chunks 2-3
            for sc in range(1, 4):
                hp_groups[hp].append((4 * sc - 6, qk_proj_closures(hp, "k", sc)))
                qdl = 4 if sc == 1 else sc * ST - 6
                hp_groups[hp].append((qdl, qk_proj_closures(hp, "q", sc)))
        hp_extras = []
        for hp in range(NHP):
            ex = []
            for dl, cls in sorted(hp_groups[hp], key=lambda g: g[0]):
                ex += [(dl + j, cl) for j, cl in enumerate(cls)]
            hp_extras.append(ex)
        for hp in range(1, NHP):
            # only what this hp's chunk0 kt0 needs rides the PREVIOUS hp's
            # tail slack: k sc0 + q sc0
            hp_extras[hp - 1] += [
                (FAR, cl) for cl in qk_proj_closures(hp, "k", 0)
            ]
            hp_extras[hp - 1] += [
                (FAR, cl) for cl in qk_proj_closures(hp, "q", 0)
            ]
        carry = []
        for hp in range(NHP):
            extras = hp_extras[hp]
            for qc in range(NQ):
                if hp < NHP - 1 and qc == 0:
                    carry = fused_chunks01(hp, extras, carry)
                    continue
                if hp < NHP - 1 and qc == 1:
                    continue  # fused into qc0 above
                if hp == NHP - 1 and qc > 0:
                    for qr in range(QC // P):
                        extras += [
                            (FAR, cl)
                            for cl in out_proj_closures((qc - 1) * (QC // P) + qr)
                        ]
                carry = attention_chunk(
                    hp, qc, extras, carry,
                    last=(hp == NHP - 1 and qc == NQ - 1),
                )
            while extras:
                extras.pop(0)[1]()
        for cl in carry:
            cl()
        # tail: all attention PSUM slots are free now -- rotate across the
        # ps_pv(3)/ps_s(2)/ps_m(1) tags so the last 8 projection groups
        # pipeline instead of serializing on a single psum drain.
        tail_tags = [
            ("ps_pv", "ps_s"),
            ("ps_m", "ps_pv"),
            ("ps_s", "ps_pv"),
            ("ps_m", "ps_s"),
        ]
        for qr in range(QC // P):
            for cl in out_proj_closures(
                (NQ - 1) * (QC // P) + qr,
                tags=tail_tags[qr],
                drains=("vector", "scalar"),
            ):
                cl()

        sb_cm.__exit__(None, None, None)
        psum_cm.__exit__(None, None, None)
        persist_cm.__exit__(None, None, None)
    nc.compile()
    return nc


_NC_CACHE = None


def _get_nc():
    global _NC_CACHE
    if _NC_CACHE is None:
        _NC_CACHE = build_kernel()
    return _NC_CACHE


def shard_inputs(inputs):
    q = np.asarray(inputs["q"], np.float32)
    k = np.asarray(inputs["k"], np.float32)
    v = np.asarray(inputs["v"], np.float32)
    Wq = np.asarray(inputs["Wq"], np.float32)
    Wk = np.asarray(inputs["Wk"], np.float32)
    Wv = np.asarray(inputs["Wv"], np.float32)
    Wo = np.asarray(inputs["Wo"], np.float32)
    bq = np.asarray(inputs["bq"], np.float32)
    bk = np.asarray(inputs["bk"], np.float32)
    bv = np.asarray(inputs["bv"], np.float32)
    bo = np.asarray(inputs["bo"], np.float32)
    assert not (bq.any() or bk.any() or bv.any()), "nonzero qkv biases unsupported"

    bf = ml_dtypes.bfloat16

    def act_layout(x):
        # x[b] is [S, D_IN]; device wants xT [D, S] staged as
        # [quarter, partition, ktile, col] with d = ktile*128 + partition
        xT = x.T  # [768, 2048]
        return np.ascontiguousarray(
            xT.reshape(KT, P, 4, S // 4).transpose(2, 1, 0, 3)
        ).astype(bf)

    def w_layout(w):
        # w slice [768, 384] -> [partition, ktile, cols]
        return np.ascontiguousarray(
            w.reshape(KT, P, DPC).transpose(1, 0, 2)
        ).astype(bf)

    in_maps = []
    for c in range(NCORES):
        b, hh = c // 2, c % 2
        cols = slice(hh * DPC, (hh + 1) * DPC)
        wo = np.ascontiguousarray(
            Wo[cols, :].reshape(HPC // 2, P, D).transpose(1, 0, 2)
        ).astype(bf)
        in_maps.append(
            {
                "qT": act_layout(q[b]),
                "kT": act_layout(k[b]),
                "vT": act_layout(v[b]),
                "wq": w_layout(Wq[:, cols]),
                "wk": w_layout(Wk[:, cols]),
                "wv": w_layout(Wv[:, cols]),
                "wo": wo,
            }
        )
    return in_maps


def gather_output(results, bo):
    out = np.empty((B, S, D), np.float32)
    for b in range(B):
        out[b] = results[2 * b]["out"].astype(np.float32) + results[
            2 * b + 1
        ]["out"].astype(np.float32)
    out += np.asarray(bo, np.float32)
    return out


def kernel(**inputs):
    from concourse.bass_utils import run_bass_kernel_spmd

    in_maps = shard_inputs(inputs)
    res = run_bass_kernel_spmd(_get_nc(), in_maps, core_ids=list(range(NCORES)))
    return gather_output(res.results, inputs["bo"])


if __name__ == "__main__":
    rng = np.random.default_rng(0)
    ins = {
        "q": rng.standard_normal((B, S, D), np.float32),
        "k": rng.standard_normal((B, S, D), np.float32),
        "v": rng.standard_normal((B, S, D), np.float32),
        "Wq": rng.standard_normal((D, D), np.float32) / np.sqrt(D),
        "bq": np.zeros(D, np.float32),
        "Wk": rng.standard_normal((D, D), np.float32) / np.sqrt(D),
        "bk": np.zeros(D, np.float32),
        "Wv": rng.standard_normal((D, D), np.float32) / np.sqrt(D),
        "bv": np.zeros(D, np.float32),
        "Wo": rng.standard_normal((D, D), np.float32) / np.sqrt(D),
        "bo": np.zeros(D, np.float32),
    }
    out = kernel(**ins)
    print("out", out.shape, out.dtype, float(np.abs(out).max()))



# revision 64
# speedup vs baseline: 1.0088x; 1.0088x over previous
"""Multi-head attention (B=4, S=2048, D=768, H=12) on 8 Trainium2 cores.

Sharding: core c handles batch b=c//2 and head-half hh=c%2 (6 of 12 heads).
Each core computes its 6 heads' contribution to out[b] = concat(O_h) @ Wo
as a partial product; the host sums the two half-head partials per batch.

Device-side layout is feature-major ("T") for q/k activations so that no
on-device transposes are needed:
  - qhT = (Wq.T @ q.T): matmul(lhsT=Wq tile, rhs=qT tile) -> [d_model, S]
    (all bf16: softmax output noise is PROPORTIONAL to score noise --
    fp8 scores measure ~5.5%% output error, over the 2e-2 gate -- so the
    whole q/k path stays bf16; the 1/sqrt(64) attention scale rides the
    exp() activation's free scale operand.)
  - S^T scores: matmul(lhsT=khT head tile, rhs=qhT head tile) -> [S_k, S_q]
    (two heads packed in the 128-row PE array: K=64 each, rows 0:64/64:128)
  - softmax: exp on ScalarE direct PSUM->SBUF (bf16); the k-sum (softmax
    denominator) comes free from a ones-column folded into the PV matmul
    stationary operand (M=65); no max-subtraction (logits are O(10) here,
    exp is safe in fp32 and the harness reference uses the same math).
  - PV: matmul(lhsT=[vh|1] tile, rhs=E^T tile) accumulated over S_k -> O^T
  - normalize: DVE copy drains the PV accumulator (frees the bank), a DMA
    shifts the denominator row to partition 0, DVE approx-reciprocal,
    GpSimd partition-broadcast, DVE multiply (+ DMA shift for the odd head).
  - out = (O^T).T @ Wo tiles -> seq-major [S, 768] partial, DMA'd out.

The ScalarE exp stream (192 x ~1.11us = 214us) is the pacing engine; the
schedule's job is to keep it gapless:
  - input DMA (~34us at the ~350GB/s per-core HBM limit) is issued in
    consumption order with host-side layouts that make every line >=4.6KB
    contiguous per partition; warm-up zero-matmuls cover the DMA waits so
    the HAM clock gate keeps the PE at 2.4GHz through the lead-in.
  - each chunk emits scores+exp first; PV lags one k-tile and the previous
    chunk's last PV + normalize ride a carry into the next chunk's slots,
    so stragglers never block the exp stream (the PE executes in order).
  - v-projections are split per head-pair (hp0's slice just-in-time in
    chunk0, hp1's in hp0's later chunks, hp2's in hp1's) and the first two
    chunks of hp0/hp1 are FUSED: qc1's first exps are borrowed into qc0's
    window and its PVs deferred to a compressed post-window, paying qc0's
    PE oversubscription out of qc1's exp-time surplus.
  - projections double-buffer one PSUM bank (ps_m x2) -- PSUM budget is
    scores 2x2 + PV 2 + proj 2 = 8 banks exactly -- which lets adjacent
    projection groups overlap their drain CASTs.
  - the final out-projection rotates over all free PSUM tags, alternates
    its drains between DVE and the (now idle) ScalarE, and the last
    chunk's normalize interleaves both halves' engine chains.
NOTE: the Tile scheduler is dependency+priority driven -- dependency-free
instructions (e.g. warm-up matmuls) get hoisted to kernel start no matter
where they are emitted; ordering tricks only work through data deps.
"""

import sys
import types

import numpy as np
import ml_dtypes

import concourse.bacc as bacc
import concourse.mybir as mybir
import concourse.tile as tile

BF16 = mybir.dt.bfloat16
FP32 = mybir.dt.float32
FP8 = mybir.dt.float8e4

B, S, D, H = 4, 2048, 768, 12
DH = 64          # head dim
HPC = 6          # heads per core
DPC = HPC * DH   # feature columns per core (384)
P = 128
KT = D // P      # 6 contraction tiles for projections
ST = S // P      # 16 seq tiles
NCORES = 8
EXP_SCALE = 1.0 / np.sqrt(DH)   # attention scale, applied for free by exp()


def _install_ntff_hook_shim():
    """The image's antenv lacks axon_hooks; provide it so trace=True works."""
    if "antenv.axon_hooks" in sys.modules:
        return
    mod = types.ModuleType("antenv.axon_hooks")
    _hook = [None]
    mod.set_axon_ntff_profile_hook = lambda h: _hook.__setitem__(0, h)
    mod.get_axon_ntff_profile_hook = lambda: _hook[0]
    sys.modules["antenv.axon_hooks"] = mod
    try:
        import antenv

        antenv.axon_hooks = mod
    except ImportError:
        pass
    try:
        from trn_agent_boot.trn_boot import _ntff_profile_via_ctypes

        mod.set_axon_ntff_profile_hook(
            _ntff_profile_via_ctypes("/opt/axon/libaxon_pjrt.so")
        )
    except Exception:
        pass


_install_ntff_hook_shim()


def build_kernel():
    nc = bacc.Bacc("TRN2", target_bir_lowering=False, debug=True)
    # activations staged host-side as [quarter, partition, ktile, col] and
    # weights as [partition, ktile, cols] so every DMA line is contiguous
    # per partition (6KB lines instead of 1KB gather lines).
    d_qT = nc.declare_dram_parameter("qT", [4, P, KT, S // 4], BF16, isOutput=False)
    d_kT = nc.declare_dram_parameter("kT", [4, P, KT, S // 4], BF16, isOutput=False)
    d_vT = nc.declare_dram_parameter("vT", [4, P, KT, S // 4], BF16, isOutput=False)
    d_wq = nc.declare_dram_parameter("wq", [P, KT, DPC], BF16, isOutput=False)
    d_wk = nc.declare_dram_parameter("wk", [P, KT, DPC], BF16, isOutput=False)
    d_wv = nc.declare_dram_parameter("wv", [P, KT, DPC], BF16, isOutput=False)
    d_wo = nc.declare_dram_parameter("wo", [P, HPC // 2, D], BF16, isOutput=False)
    # Wo rows for hp2's ODD head, staged at partitions 0:64: lets the tail
    # out-projection read the odd-head O straight from the normalize's
    # o_tmp tile (partitions 0:64) instead of waiting for the shift DMA
    d_wo_lo = nc.declare_dram_parameter("wo_lo", [DH, D], BF16, isOutput=False)
    d_out = nc.declare_dram_parameter("out", [S, D], BF16, isOutput=True)

    with tile.TileContext(nc) as tc:
        persist_cm = tc.tile_pool(name="persist", bufs=1)
        pp = persist_cm.__enter__()

        # --- persistent SBUF inputs ---
        sb_qT = pp.tile([P, KT, S], BF16, tag="sb_qT")
        sb_kT = pp.tile([P, KT, S], BF16, tag="sb_kT")
        sb_vT = pp.tile([P, KT, S], BF16, tag="sb_vT")
        sb_wq = pp.tile([P, KT, DPC], BF16, tag="sb_wq")
        sb_wk = pp.tile([P, KT, DPC], BF16, tag="sb_wk")
        sb_wv = pp.tile([P, KT, DPC], BF16, tag="sb_wv")
        sb_wo = pp.tile([P, HPC // 2, D], BF16, tag="sb_wo")
        sb_wo_lo = pp.tile([DH, D], BF16, tag="sb_wo_lo")
        warm_sb = pp.tile([P, 512], BF16, tag="warm_sb")
        ones_pb = pp.tile([P, DH], FP32, tag="ones_pb")
        nc.vector.memset(ones_pb, 1.0)

        # PE warm-up: the HAM clock gate needs ~3.4us of sustained matmul
        # activity to lift the PE from 1.2 to 2.4 GHz.  Burn it on zero
        # matmuls while the first input DMAs are still in flight.
        nc.gpsimd.memset(warm_sb, 0.0)
        psum_cm = tc.tile_pool(name="ps", bufs=1, space="PSUM")
        psm = psum_cm.__enter__()
        warm_ps = psm.tile([P, 2, 512], FP32, tag="ps_s", name="warm_ps", bufs=2)
        for i in range(16):
            nc.tensor.matmul(
                warm_ps[:, i % 2, :], warm_sb[:, 0:P], warm_sb, start=True, stop=True
            )

        # --- input DMA in consumption order ---
        # Weights first (single launches, >=512B lines), then the first
        # column chunk of kT/qT/vT so projections can start, then the rest
        # of kT/vT interleaved (chunk (hp0,qc0) consumes ALL of kh and vh),
        # qT trailing (chunk qc only needs qh columns qc*512+), wo last.
        def dma_w(sb, dr):
            nc.sync.dma_start(out=sb, in_=dr[:, :, :])

        def dma_cols(sb, dr, i):
            w = S // 4
            nc.sync.dma_start(
                out=sb[:, :, i * w : (i + 1) * w], in_=dr[i, :, :, :]
            )

        # critical path to the FIRST exp: wk+kT0 (k-proj sc0), wq+qT0
        # (q-proj sc0) -- everything else trails in consumption order.
        dma_w(sb_wk, d_wk)
        dma_cols(sb_kT, d_kT, 0)
        dma_w(sb_wq, d_wq)
        dma_cols(sb_qT, d_qT, 0)
        dma_w(sb_wv, d_wv)
        dma_cols(sb_vT, d_vT, 0)
        dma_cols(sb_kT, d_kT, 1)
        dma_cols(sb_vT, d_vT, 1)
        dma_cols(sb_qT, d_qT, 1)
        dma_cols(sb_kT, d_kT, 2)
        dma_cols(sb_vT, d_vT, 2)
        dma_cols(sb_kT, d_kT, 3)
        dma_cols(sb_vT, d_vT, 3)
        dma_cols(sb_qT, d_qT, 2)
        dma_cols(sb_qT, d_qT, 3)
        nc.sync.dma_start(out=sb_wo, in_=d_wo[:, :, :])
        nc.sync.dma_start(out=sb_wo_lo, in_=d_wo_lo[:, :])

        QC = 512           # q positions per attention chunk
        NQ = S // QC       # 4 chunks
        NC2 = D // 2       # output projection n-halves (one PSUM bank each)
        NHP = HPC // 2     # 3 head pairs
        FAR = 10**9

        # --- persistent activations ---
        sb_qh = [pp.tile([P, S], BF16, tag=f"sb_qh{i}", name=f"sb_qh{i}") for i in range(NHP)]
        sb_kh = [pp.tile([P, S], BF16, tag=f"sb_kh{i}", name=f"sb_kh{i}") for i in range(NHP)]
        sb_vh = [
            pp.tile([P, HPC, DH + 1], BF16, tag=f"sb_vh{i}", name=f"sb_vh{i}")
            for i in range(ST)
        ]  # [v | 1] per seq tile
        sb_o = [
            [pp.tile([P, QC], BF16, tag=f"sb_o{i}_{j}", name=f"sb_o{i}_{j}") for j in range(NQ)]
            for i in range(NHP)
        ]
        for i in range(ST):
            nc.vector.memset(sb_vh[i][:, :, DH : DH + 1], 1.0)

        sb_cm = tc.tile_pool(name="work", bufs=1)
        wk = sb_cm.__enter__()

        def v_proj_closures(st, part):
            """part = head-pair: hp0's slice is needed in chunk0, hp1's
            weaves into hp0's chunks 1-3, hp2's into hp1's chunks 1-3.
            Splitting keeps each chunk's extra PE load bounded."""
            c0, c1 = 2 * DH * part, 2 * DH * (part + 1)
            nh = (c1 - c0) // DH
            stt = {}
            out = []
            for kt in range(KT):
                def mm(kt=kt):
                    if kt == 0:
                        stt["ps"] = psm.tile(
                            [P, c1 - c0], FP32, tag="ps_m", name="ps_v"
                        )
                    nc.tensor.matmul(
                        stt["ps"],
                        sb_vT[:, kt, st * P : (st + 1) * P],
                        sb_wv[:, kt, c0:c1],
                        start=(kt == 0),
                        stop=(kt == KT - 1),
                    )
                    if kt == KT - 1:
                        nc.vector.tensor_copy(
                            out=sb_vh[st][:, c0 // DH : c1 // DH, 0:DH],
                            in_=stt["ps"][:].rearrange("p (h d) -> p h d", h=nh),
                        )
                out.append(mm)
            return out

        def qk_proj_closures(hp, which, sc):
            """One closure per PE instruction so the group can be woven
            between attention k-tiles (PE executes its stream in order)."""
            sb_w, sb_x, dst = (
                (sb_wq, sb_qT, sb_qh[hp]) if which == "q" else (sb_wk, sb_kT, sb_kh[hp])
            )
            st = {}
            out = []
            for kt in range(KT):
                def mm(kt=kt):
                    if kt == 0:
                        st["ps"] = psm.tile([P, 512], FP32, tag="ps_m", name="ps_qk")
                    nc.tensor.matmul(
                        st["ps"],
                        sb_w[:, kt, hp * P : (hp + 1) * P],
                        sb_x[:, kt, sc * 512 : (sc + 1) * 512],
                        start=(kt == 0),
                        stop=(kt == KT - 1),
                    )
                    if kt == KT - 1:
                        nc.vector.tensor_copy(
                            out=dst[:, sc * 512 : (sc + 1) * 512], in_=st["ps"]
                        )
                out.append(mm)
            return out

        last_otmp = [None]

        def out_proj_closures(qt, tags=("ps_m", "ps_m"), drains=("vector", "vector")):
            qc, qr = qt // (QC // P), qt % (QC // P)
            # for the LAST chunk, hp2's odd head is read straight from the
            # normalize's o_tmp tile (partitions 0:64, against wo_lo) so the
            # tail never waits for the partition-shift DMA
            split2 = qc == NQ - 1 and last_otmp[0] is not None
            st = {}
            out = []
            for n2 in range(2):
                for hp in range(NHP):
                    def mm(n2=n2, hp=hp):
                        if hp == 0 and n2 == 0:
                            st["outt"] = wk.tile([P, D], BF16, tag="outt", bufs=4, name="outt")
                        if hp == 0:
                            bufs = {"ps_pv": 2, "ps_s": 2, "ps_m": 2}[tags[n2]]
                            st["ps"] = psm.tile(
                                [P, NC2], FP32, tag=tags[n2], name="ps_o", bufs=bufs
                            )
                        if hp == NHP - 1 and split2:
                            nc.tensor.matmul(
                                st["ps"],
                                sb_o[hp][qc][0:DH, qr * P : (qr + 1) * P],
                                sb_wo[0:DH, hp, n2 * NC2 : (n2 + 1) * NC2],
                                start=False,
                                stop=False,
                            )
                            nc.tensor.matmul(
                                st["ps"],
                                last_otmp[0][:, qr * P : (qr + 1) * P],
                                sb_wo_lo[:, n2 * NC2 : (n2 + 1) * NC2],
                                start=False,
                                stop=True,
                            )
                        else:
                            nc.tensor.matmul(
                                st["ps"],
                                sb_o[hp][qc][:, qr * P : (qr + 1) * P],
                                sb_wo[:, hp, n2 * NC2 : (n2 + 1) * NC2],
                                start=(hp == 0),
                                stop=(hp == NHP - 1),
                            )
                        if hp == NHP - 1:
                            if drains[n2] == "scalar":
                                # ScalarE is idle after the last exp; let it
                                # drain half the tail psum groups in parallel
                                nc.scalar.copy(
                                    out=st["outt"][:, n2 * NC2 : (n2 + 1) * NC2],
                                    in_=st["ps"],
                                )
                            else:
                                nc.vector.tensor_copy(
                                    out=st["outt"][:, n2 * NC2 : (n2 + 1) * NC2],
                                    in_=st["ps"],
                                )
                            # ship each half as soon as it is copied
                            nc.sync.dma_start(
                                out=d_out[
                                    qt * P : (qt + 1) * P,
                                    n2 * NC2 : (n2 + 1) * NC2,
                                ],
                                in_=st["outt"][:, n2 * NC2 : (n2 + 1) * NC2],
                            )
                    out.append(mm)
            return out

        def alloc_ps_pv():
            return [
                psm.tile([P, QC], FP32, tag="ps_pv", name="ps_pv_e", bufs=2),
                psm.tile([P, QC], FP32, tag="ps_pv", name="ps_pv_o", bufs=2),
            ]

        def scores_exp(hp, qc, kt):
            ps_s = psm.tile([P, 2, QC], FP32, tag="ps_s", name="ps_s", bufs=2)
            for h01 in range(2):
                hs = slice(DH * h01, DH * (h01 + 1))
                nc.tensor.matmul(
                    ps_s[:, h01, :],
                    sb_kh[hp][hs, kt * P : kt * P + P],
                    sb_qh[hp][hs, qc * QC : qc * QC + QC],
                    start=True,
                    stop=True,
                    tile_position=(DH * h01, 0),
                )
            e_t = wk.tile([P, 2, QC], BF16, tag="e_t", bufs=12, name="e_t")
            nc.scalar.activation(
                out=e_t,
                in_=ps_s,
                func=mybir.ActivationFunctionType.Exp,
                scale=float(EXP_SCALE),
            )
            return e_t

        def pv_pair(ps_pv, hp, kt, e_t):
            for h01 in range(2):
                h = hp * 2 + h01
                nc.tensor.matmul(
                    ps_pv[h01][0 : DH + 1, :],
                    sb_vh[kt][:, h, :],
                    e_t[:, h01, :],
                    start=(kt == 0),
                    stop=(kt == ST - 1),
                )

        def normalize_half(ps_pv, hp, qc, h01):
            # normalize: O^T[d, q] / denom[q]; denom sits at PSUM row DH.
            # One full copy off PSUM releases the accumulator slot early; the
            # rest of the chain works from SBUF off the critical path.
            o_un = wk.tile([DH + 1, QC], FP32, tag="o_un", bufs=4, name="o_un")
            nc.vector.tensor_copy(out=o_un, in_=ps_pv[h01][0 : DH + 1, :])
            rts = wk.tile([1, QC], FP32, tag="rts", bufs=2, name="rts")
            nc.sync.dma_start(out=rts, in_=o_un[DH : DH + 1, :])
            rt0 = wk.tile([1, QC], FP32, tag="rt0", bufs=2, name="rt0")
            nc.vector.reciprocal_approx_fast(out=rt0, in_=rts)
            bcr = wk.tile([DH, QC], FP32, tag="bcr", bufs=2, name="bcr")
            nc.gpsimd.partition_broadcast(bcr, rt0, channels=DH)
            if h01 == 0:
                nc.vector.tensor_mul(
                    out=sb_o[hp][qc][0:DH, :], in0=o_un[0:DH, :], in1=bcr
                )
            else:
                # odd head belongs at partitions 64:128 of the pair-packed
                # O^T; DVE can't cross lanes, so temp tile + DMA shift.
                o_tmp = wk.tile([DH, QC], BF16, tag="o_tmp", bufs=2, name="o_tmp")
                nc.vector.tensor_mul(out=o_tmp, in0=o_un[0:DH, :], in1=bcr)
                nc.sync.dma_start(out=sb_o[hp][qc][DH:P, :], in_=o_tmp)

        def normalize_both(ps_pv, hp, qc):
            """Interleaved both-halves normalize for the LAST chunk: the two
            chains' DVE/gpsimd/DMA stages alternate so neither engine queue
            serializes the other half behind it (~1.7us faster than two
            sequential normalize_half calls)."""
            o_un_e = wk.tile([DH + 1, QC], FP32, tag="o_un", bufs=4, name="o_un")
            nc.vector.tensor_copy(out=o_un_e, in_=ps_pv[0][0 : DH + 1, :])
            o_un_o = wk.tile([DH + 1, QC], FP32, tag="o_un", bufs=4, name="o_un")
            nc.vector.tensor_copy(out=o_un_o, in_=ps_pv[1][0 : DH + 1, :])
            rts_e = wk.tile([1, QC], FP32, tag="rts", bufs=2, name="rts")
            nc.sync.dma_start(out=rts_e, in_=o_un_e[DH : DH + 1, :])
            rts_o = wk.tile([1, QC], FP32, tag="rts", bufs=2, name="rts")
            nc.sync.dma_start(out=rts_o, in_=o_un_o[DH : DH + 1, :])
            rt0_e = wk.tile([1, QC], FP32, tag="rt0", bufs=2, name="rt0")
            nc.vector.reciprocal_approx_fast(out=rt0_e, in_=rts_e)
            rt0_o = wk.tile([1, QC], FP32, tag="rt0", bufs=2, name="rt0")
            nc.vector.reciprocal_approx_fast(out=rt0_o, in_=rts_o)
            bcr_e = wk.tile([DH, QC], FP32, tag="bcr", bufs=2, name="bcr")
            nc.gpsimd.partition_broadcast(bcr_e, rt0_e, channels=DH)
            bcr_o = wk.tile([DH, QC], FP32, tag="bcr", bufs=2, name="bcr")
            nc.gpsimd.partition_broadcast(bcr_o, rt0_o, channels=DH)
            nc.vector.tensor_mul(
                out=sb_o[hp][qc][0:DH, :], in0=o_un_e[0:DH, :], in1=bcr_e
            )
            o_tmp = wk.tile([DH, QC], BF16, tag="o_tmp", bufs=2, name="o_tmp")
            nc.vector.tensor_mul(out=o_tmp, in0=o_un_o[0:DH, :], in1=bcr_o)
            # no shift DMA here: the tail out-projection reads o_tmp
            # directly (split hp2 contraction against wo_lo)
            last_otmp[0] = o_tmp

        def attention_chunk(hp, qc, extras, carry, last=False):
            """Emits this chunk's scores/exp stream; the previous chunk's
            last PV + normalize halves arrive via `carry` and are emitted
            after this chunk's first exps so the exp stream never waits on
            them. Returns this chunk's own carry list."""
            ps_pv = alloc_ps_pv()
            pv_prev = [None]
            for kt in range(ST):
                gkt = qc * ST + kt
                e_t = scores_exp(hp, qc, kt)
                # previous chunk's straggler work (its last PV + normalize),
                # one piece per slot right after the exp is queued
                if carry:
                    carry.pop(0)()
                # deadline-due producers emit AFTER this slot's exp, so a
                # stalled producer (DMA-gated projection) never delays the
                # exp stream; their readers are >=1 slot downstream.
                while extras and extras[0][0] <= gkt:
                    extras.pop(0)[1]()
                # PV lags scores/exp by one k-tile: a late vh[kt] write
                # stalls only the (off-critical-path) accumulate, not exp.
                if pv_prev[0] is not None:
                    pv_prev[0]()
                pv_prev[0] = lambda kt=kt, e=e_t: pv_pair(ps_pv, hp, kt, e)
                # weave in foreign PE work (next projections / output proj)
                # where the PE has slack relative to the exp stream
                budget = 1
                while extras and budget > 0:
                    extras.pop(0)[1]()
                    budget -= 1
            if last:
                def last_pv_and_norm(f=pv_prev[0]):
                    f()
                    normalize_both(ps_pv, hp, qc)

                return [last_pv_and_norm]

            def last_pv_and_norm_e(f=pv_prev[0]):
                f()
                normalize_half(ps_pv, hp, qc, 0)

            return [
                last_pv_and_norm_e,
                lambda: normalize_half(ps_pv, hp, qc, 1),
            ]

        BOR = 7

        def fused_chunks01(hp, extras, carry):
            """qc0+qc1 fused: qc1's first BOR exps are emitted inside qc0's
            slots ST-BOR..ST-1 and its remaining exps + ALL its PVs in a
            compressed post window. qc0's PE oversubscription (v-proj +
            k-proj + its own attention exceed its exp time) is paid from
            qc1's exp-time surplus instead of stalling the exp stream."""
            ps_pv0 = alloc_ps_pv()
            e1 = []
            pv_prev = [None]
            for kt in range(ST):
                e_t0 = scores_exp(hp, 0, kt)
                if carry:
                    carry.pop(0)()
                while extras and extras[0][0] <= kt:
                    extras.pop(0)[1]()
                if pv_prev[0] is not None:
                    pv_prev[0]()
                pv_prev[0] = lambda kt=kt, e=e_t0: pv_pair(ps_pv0, hp, kt, e)
                if kt >= ST - BOR:
                    e1.append(scores_exp(hp, 1, kt - (ST - BOR)))
                budget = 1
                while extras and budget > 0:
                    extras.pop(0)[1]()
                    budget -= 1
            def last_pv0_and_norm_e(f=pv_prev[0]):
                f()
                normalize_half(ps_pv0, hp, 0, 0)

            carry0 = [
                last_pv0_and_norm_e,
                lambda: normalize_half(ps_pv0, hp, 0, 1),
            ]
            ps_pv1 = alloc_ps_pv()
            pend = 0
            for s in range(ST - BOR):
                kt = BOR + s
                e1.append(scores_exp(hp, 1, kt))
                if carry0:
                    carry0.pop(0)()
                gk = ST + 2 * s
                while extras and extras[0][0] <= gk:
                    extras.pop(0)[1]()
                for _ in range(2):
                    if pend < ST and pend <= kt and pend < len(e1):
                        pv_pair(ps_pv1, hp, pend, e1[pend])
                        pend += 1
            while pend < ST:
                pv_pair(ps_pv1, hp, pend, e1[pend])
                pend += 1
            while carry0:
                carry0.pop(0)()
            return [
                lambda: normalize_half(ps_pv1, hp, 1, 0),
                lambda: normalize_half(ps_pv1, hp, 1, 1),
            ]

        # --- schedule: front-load only what attention chunk (0,0) needs at
        # its very first k-tiles (early v-proj tiles, first k/q projection
        # chunks of hp0); everything else is woven into the attention chunks'
        # PE slack with deadlines that keep writers ahead of their readers.
        for cl in qk_proj_closures(0, "k", 0):
            cl()
        # more zero matmuls between k-proj (kT0-gated) and q-proj
        # (qT0-gated): they fill the DMA wait so the HAM clock gate never
        # sees a >3.4us PE idle and q-proj runs at 2.4GHz, not 1.2
        for i in range(8):
            nc.tensor.matmul(
                warm_ps[:, i % 2, :], warm_sb[:, 0:P], warm_sb, start=True, stop=True
            )
        for cl in qk_proj_closures(0, "q", 0):
            cl()
        # groups woven into each hp's chunks, each kept contiguous in list
        # order (they share one PSUM slot); deadlines are local k-tile slots
        hp_groups = [[] for _ in range(NHP)]
        for st in range(ST):
            # the vh[st] copy (last closure) must land by slot st+1 where
            # PV(st) reads it; hp0's 2-head slice only
            hp_groups[0].append((st - 4, v_proj_closures(st, 0)))
            # hp1's 2-head slice: first half rides hp0's chunks 1-3 slack,
            # second half lands just-in-time in hp1's own chunk0 (hp0 is
            # PE-oversubscribed; hp1 has slack). hp2's slice rides hp1's
            # chunks 1-3.
            if st < 8:
                hp_groups[0].append((16 + 4 * st, v_proj_closures(st, 1)))
            else:
                hp_groups[1].append((st - 4, v_proj_closures(st, 1)))
            hp_groups[1].append((18 + 2 * st, v_proj_closures(st, 2)))
        for sc in range(1, 4):
            hp_groups[0].append((4 * sc - 6, qk_proj_closures(0, "k", sc)))
            # q sc1 must finish by slot ST-BOR-1 where the fused chunk0
            # starts borrowing qc1's scores (qT1 has landed by ~slot 4)
            qdl = 4 if sc == 1 else sc * ST - 6
            hp_groups[0].append((qdl, qk_proj_closures(0, "q", sc)))
        for hp in range(1, NHP):
            # k sc1-3 are due just before this hp's chunk0 reads kt=4sc;
            # q sc1 early (fused chunk0 borrows qc1 scores from slot ST-BOR),
            # q sc2-3 before its chunks 2-3
            for sc in range(1, 4):
                hp_groups[hp].append((4 * sc - 6, qk_proj_closures(hp, "k", sc)))
                qdl = 4 if sc == 1 else sc * ST - 6
                hp_groups[hp].append((qdl, qk_proj_closures(hp, "q", sc)))
        hp_extras = []
        for hp in range(NHP):
            ex = []
            for dl, cls in sorted(hp_groups[hp], key=lambda g: g[0]):
                ex += [(dl + j, cl) for j, cl in enumerate(cls)]
            hp_extras.append(ex)
        for hp in range(1, NHP):
            # only what this hp's chunk0 kt0 needs rides the PREVIOUS hp's
            # tail slack: k sc0 + q sc0
            hp_extras[hp - 1] += [
                (FAR, cl) for cl in qk_proj_closures(hp, "k", 0)
            ]
            hp_extras[hp - 1] += [
                (FAR, cl) for cl in qk_proj_closures(hp, "q", 0)
            ]
        carry = []
        for hp in range(NHP):
            extras = hp_extras[hp]
            for qc in range(NQ):
                if hp < NHP - 1 and qc == 0:
                    carry = fused_chunks01(hp, extras, carry)
                    continue
                if hp < NHP - 1 and qc == 1:
                    continue  # fused into qc0 above
                if hp == NHP - 1 and qc > 0:
                    for qr in range(QC // P):
                        extras += [
                            (FAR, cl)
                            for cl in out_proj_closures((qc - 1) * (QC // P) + qr)
                        ]
                carry = attention_chunk(
                    hp, qc, extras, carry,
                    last=(hp == NHP - 1 and qc == NQ - 1),
                )
            while extras:
                extras.pop(0)[1]()
        for cl in carry:
            cl()
        # tail: all attention PSUM slots are free now -- rotate across the
        # ps_pv(3)/ps_s(2)/ps_m(1) tags so the last 8 projection groups
        # pipeline instead of serializing on a single psum drain.
        tail_tags = [
            ("ps_pv", "ps_s"),
            ("ps_m", "ps_pv"),
            ("ps_s", "ps_pv"),
            ("ps_m", "ps_s"),
        ]
        for qr in range(QC // P):
            for cl in out_proj_closures(
                (NQ - 1) * (QC // P) + qr,
                tags=tail_tags[qr],
                drains=("vector", "scalar"),
            ):
                cl()

        sb_cm.__exit__(None, None, None)
        psum_cm.__exit__(None, None, None)
        persist_cm.__exit__(None, None, None)
    nc.compile()
    return nc


_NC_CACHE = None


def _get_nc():
    global _NC_CACHE
    if _NC_CACHE is None:
        _NC_CACHE = build_kernel()
    return _NC_CACHE


def shard_inputs(inputs):
    q = np.asarray(inputs["q"], np.float32)
    k = np.asarray(inputs["k"], np.float32)
    v = np.asarray(inputs["v"], np.float32)
    Wq = np.asarray(inputs["Wq"], np.float32)
    Wk = np.asarray(inputs["Wk"], np.float32)
    Wv = np.asarray(inputs["Wv"], np.float32)
    Wo = np.asarray(inputs["Wo"], np.float32)
    bq = np.asarray(inputs["bq"], np.float32)
    bk = np.asarray(inputs["bk"], np.float32)
    bv = np.asarray(inputs["bv"], np.float32)
    bo = np.asarray(inputs["bo"], np.float32)
    assert not (bq.any() or bk.any() or bv.any()), "nonzero qkv biases unsupported"

    bf = ml_dtypes.bfloat16

    def act_layout(x):
        # x[b] is [S, D_IN]; device wants xT [D, S] staged as
        # [quarter, partition, ktile, col] with d = ktile*128 + partition
        xT = x.T  # [768, 2048]
        return np.ascontiguousarray(
            xT.reshape(KT, P, 4, S // 4).transpose(2, 1, 0, 3)
        ).astype(bf)

    def w_layout(w):
        # w slice [768, 384] -> [partition, ktile, cols]
        return np.ascontiguousarray(
            w.reshape(KT, P, DPC).transpose(1, 0, 2)
        ).astype(bf)

    in_maps = []
    for c in range(NCORES):
        b, hh = c // 2, c % 2
        cols = slice(hh * DPC, (hh + 1) * DPC)
        wo = np.ascontiguousarray(
            Wo[cols, :].reshape(HPC // 2, P, D).transpose(1, 0, 2)
        ).astype(bf)
        in_maps.append(
            {
                "qT": act_layout(q[b]),
                "kT": act_layout(k[b]),
                "vT": act_layout(v[b]),
                "wq": w_layout(Wq[:, cols]),
                "wk": w_layout(Wk[:, cols]),
                "wv": w_layout(Wv[:, cols]),
                "wo": wo,
                # hp2's odd-head Wo rows at partitions 0:64 (tail split)
                "wo_lo": np.ascontiguousarray(wo[DH:P, 2, :]),
            }
        )
    return in_maps


def gather_output(results, bo):
    out = np.empty((B, S, D), np.float32)
    for b in range(B):
        out[b] = results[2 * b]["out"].astype(np.float32) + results[
            2 * b + 1
        ]["out"].astype(np.float32)
    out += np.asarray(bo, np.float32)
    return out


def kernel(**inputs):
    from concourse.bass_utils import run_bass_kernel_spmd

    in_maps = shard_inputs(inputs)
    res = run_bass_kernel_spmd(_get_nc(), in_maps, core_ids=list(range(NCORES)))
    return gather_output(res.results, inputs["bo"])


if __name__ == "__main__":
    rng = np.random.default_rng(0)
    ins = {
        "q": rng.standard_normal((B, S, D), np.float32),
        "k": rng.standard_normal((B, S, D), np.float32),
        "v": rng.standard_normal((B, S, D), np.float32),
        "Wq": rng.standard_normal((D, D), np.float32) / np.sqrt(D),
        "bq": np.zeros(D, np.float32),
        "Wk": rng.standard_normal((D, D), np.float32) / np.sqrt(D),
        "bk": np.zeros(D, np.float32),
        "Wv": rng.standard_normal((D, D), np.float32) / np.sqrt(D),
        "bv": np.zeros(D, np.float32),
        "Wo": rng.standard_normal((D, D), np.float32) / np.sqrt(D),
        "bo": np.zeros(D, np.float32),
    }
    out = kernel(**ins)
    print("out", out.shape, out.dtype, float(np.abs(out).max()))

